# revision 19
# baseline (speedup 1.0000x reference)
"""Trainium2 Bass kernel for nn_DeepLatent loss (chamfer + L2 of a per-point MLP).

Strategy (8 cores, data-parallel over batch B=32 -> 4 samples/core):
  Per core, per sample s (channel-major layout: activations stored [C, Npoints]):
    h1 = relu(W1o.T @ obs^T + latbias)        latbias = W1lat.T @ latent + b1 (tiny matmul)
    h2 = relu(W2.T @ h1 + b2)
    h3 = relu(W3.T @ h2 + b3)
    delta = W4.T @ h3                         est = obs + delta + b4
  Chamfer via augmented grams (K=6 matmuls: 3 coord rows + 3 aux rows):
    G [n,m]  = gt_n . est_m - |est_m|^2/2     (aux lhsT rows = -0.5, aux rhs rows = est^2)
    G'[m,n]  = est_m . gt_n - |gt_n|^2/2
    min_m d2[n,m] = |gt_n|^2 - 2 max_m G[n,m]   (max via fused DVE tensor_tensor_reduce)
  Per-core partial sums (max-sums, sq-sums, cross-sum) are combined on the host.

All matmuls use float32r (fp22 truncation, 1 cycle/col at free-dim>=256).
"""

import numpy as np
from contextlib import ExitStack

import concourse.bass as bass
import concourse.bacc as bacc
import concourse.mybir as mybir
import concourse.tile as tile
from concourse.bass_utils import run_bass_kernel_spmd

F32 = mybir.dt.float32
F32R = mybir.dt.float32r
AX = mybir.AxisListType
OP = mybir.AluOpType
ACTF = mybir.ActivationFunctionType

B, N, L = 32, 1024, 256
NCORES = 8
BS = B // NCORES  # samples per core
NT = N // 128     # n-tiles per sample
NEG = -3.0e38

# test.py hooks
TRACE = False
LAST = None


def _r(ap):
    return ap.bitcast(F32R)


def build_program(do_mlp=True, do_gram=True):
    nc = bacc.Bacc()

    # host-pretransposed layouts: every DMA below is inner-contiguous
    obs_d = nc.dram_tensor("obs_t", [3, BS, N], F32, kind="ExternalInput")[:]
    gt_d = nc.dram_tensor("gt_t", [3, BS, N], F32, kind="ExternalInput")[:]
    lat_d = nc.dram_tensor("lat_t", [L, BS], F32, kind="ExternalInput")[:]
    W1od = nc.dram_tensor("w1o", [3, 512], F32, kind="ExternalInput")[:]
    W1ld = nc.dram_tensor("w1l", [128, 2, 512], F32, kind="ExternalInput")[:]
    b1d = nc.dram_tensor("b1r", [1, 512], F32, kind="ExternalInput")[:]
    W2d = nc.dram_tensor("w2p", [128, 4, 512], F32, kind="ExternalInput")[:]
    b2d = nc.dram_tensor("b2p", [128, 4], F32, kind="ExternalInput")[:]
    W3d = nc.dram_tensor("w3p", [128, 4, 256], F32, kind="ExternalInput")[:]
    b3d = nc.dram_tensor("b3p", [128, 2], F32, kind="ExternalInput")[:]
    W4d = nc.dram_tensor("w4p", [128, 2, 3], F32, kind="ExternalInput")[:]
    b4d = nc.dram_tensor("b4p", [3, 1], F32, kind="ExternalInput")[:]
    out_d = nc.dram_tensor("partials", [1, 8], F32, kind="ExternalOutput")[:]

    with tile.TileContext(nc) as tc, ExitStack() as ctx:
        singles = ctx.enter_context(tc.tile_pool(name="singles", bufs=1))

        def fixed(shape, name):
            return singles.tile(shape, F32, tag=name, name=name)

        # ---------- fixed tiles ----------
        w1o = fixed([3, 512], "w1o")
        w1l = fixed([128, 2, 512], "w1l")
        b1r = fixed([1, 512], "b1r")
        w2t = fixed([128, 4, 512], "w2t")
        w3t = fixed([128, 4, 256], "w3t")
        w4t = fixed([128, 2, 3], "w4t")
        b2t = fixed([128, 4], "b2t")
        b3t = fixed([128, 2], "b3t")
        b4p = fixed([3, 1], "b4p")
        latT = fixed([128, 2, BS], "latT")
        ones_r = fixed([1, BS], "ones_r")
        ones_c = fixed([128, 1], "ones_c")
        latb = fixed([128, 4, BS], "latb")
        Pg = fixed([3, BS, N], "Pg")
        Pe = fixed([3, BS, N], "Pe")
        Pg2s = fixed([3, N], "Pg2s")
        Pe2s = fixed([3, N], "Pe2s")
        SGS = fixed([3, BS], "SGS")
        neghalf = fixed([3, N], "neghalf")
        M1 = fixed([128, BS * NT], "M1")
        M2 = fixed([128, BS * NT], "M2")
        Ft = fixed([128, 8], "Ft")
        SES = fixed([3, BS], "SES")
        ttr_dump = fixed([128, 512], "ttr_dump")
        outs = fixed([1, 8], "outs")
        A_ = [fixed([128, N], f"Areg{i}") for i in range(2)]
        B_ = [fixed([128, N], f"Breg{i}") for i in range(2)]
        C_ = [fixed([128, N], f"Creg{i}") for i in range(2)]
        D_ = [fixed([128, N], f"Dreg{i}") for i in range(2)]

        h1p = ctx.enter_context(tc.tile_pool(name="h1", bufs=2))
        h2p = ctx.enter_context(tc.tile_pool(name="h2", bufs=2))
        h3p = ctx.enter_context(tc.tile_pool(name="h3", bufs=2))
        otp = ctx.enter_context(tc.tile_pool(name="obsT", bufs=2))
        psA = ctx.enter_context(tc.tile_pool(name="psA", bufs=2, space="PSUM"))
        psG = ctx.enter_context(tc.tile_pool(name="psG", bufs=2, space="PSUM"))

        # ---------- startup ----------
        nc.sync.dma_start(out=_r(w1o[:, :]), in_=_r(W1od))
        nc.sync.dma_start(out=w1l, in_=W1ld)
        nc.sync.dma_start(out=b1r, in_=b1d)
        nc.sync.dma_start(out=_r(w2t[:, :, :]), in_=_r(W2d))
        nc.sync.dma_start(out=_r(w3t[:, :, :]), in_=_r(W3d))
        nc.sync.dma_start(out=_r(w4t[:, :, :]), in_=_r(W4d))
        nc.sync.dma_start(out=b2t, in_=b2d)
        nc.sync.dma_start(out=b3t, in_=b3d)
        nc.sync.dma_start(out=b4p, in_=b4d)
        for k in range(2):
            nc.sync.dma_start(out=latT[:, k, :], in_=lat_d[128 * k:128 * (k + 1), :])
        nc.sync.dma_start(out=_r(Pg[:, :, :]), in_=_r(gt_d))
        nc.vector.memset(ones_r, 1.0)
        nc.vector.memset(ones_c, 1.0)
        nc.vector.memset(Ft, 0.0)
        # aux lhsT rows {3-5, 35-37} of A/B must be -0.5: memset an fp32
        # staging row-band, then DMA it in (f32r-tagged) since compute engines
        # cannot emit float32r directly.
        nc.vector.memset(neghalf, -0.5)
        for t_ in A_ + B_:
            for g in range(2):
                nc.gpsimd.dma_start(out=_r(t_[32 * g + 3:32 * g + 6, :]),
                                    in_=_r(neghalf[:, :]))

        # latent bias vectors: latb[cout, c-tile, s] = (latent @ W1[3:] + b1)^T
        for c in range(4):
            lps = psG.tile([128, 1024], F32, tag="g", name=f"latps{c}")
            for k in range(2):
                nc.tensor.matmul(lps[:, 0:BS], w1l[:, k, 128 * c:128 * (c + 1)],
                                 latT[:, k, :], start=(k == 0), stop=False)
            nc.tensor.matmul(lps[:, 0:BS], b1r[:, 128 * c:128 * (c + 1)],
                             ones_r[:, :], start=False, stop=True)
            nc.vector.tensor_copy(latb[:, c, :], lps[:, 0:BS])

        # ---------- per-sample gram rounds (generator; interleaved with next MLP) ----------
        def gram_rounds(s):
            Ar, Br, Cr, Dr = A_[s % 2], B_[s % 2], C_[s % 2], D_[s % 2]
            for lhs_reg, rhs_reg, Mt in ((Ar, Cr, M1), (Br, Dr, M2)):
                for r in range(4):
                    gtiles = []
                    for g in range(2):
                        t = 2 * r + g
                        gp = psG.tile([128, 1024], F32, tag="g", name=f"gp{s}_{r}_{g}")
                        for j in range(2):
                            nc.tensor.matmul(
                                gp[:, 512 * j:512 * (j + 1)],
                                _r(lhs_reg[32 * g:32 * g + 6, 128 * t:128 * (t + 1)]),
                                _r(rhs_reg[32 * g:32 * g + 6, 512 * j:512 * (j + 1)]),
                                start=True, stop=True)
                        gtiles.append((t, gp))
                    for t, gp in gtiles:
                        nc.vector.tensor_reduce(
                            out=Mt[:, NT * s + t:NT * s + t + 1], in_=gp[:, :],
                            axis=AX.X, op=OP.max)
                    yield

        def advance(it):
            if it is not None:
                next(it, None)

        # ---------- per-sample MLP ----------
        def mlp(s, hooks):
            obsT = otp.tile([3, N], F32, tag="obsT", name=f"obsT{s}")
            nc.gpsimd.dma_start(out=_r(obsT[:, :]), in_=_r(obs_d[:, s, :]))
            nc.scalar.activation(_r(Pg2s[:, :]), Pg[:, s, :], ACTF.Square,
                                 accum_out=SGS[:, s:s + 1])
            Ar, Dr = A_[s % 2], D_[s % 2]
            for g in range(2):
                nc.gpsimd.dma_start(out=_r(Ar[32 * g:32 * g + 3, :]), in_=_r(Pg[:, s, :]))
                nc.gpsimd.dma_start(out=_r(Dr[32 * g:32 * g + 3, :]), in_=_r(Pg[:, s, :]))
                nc.gpsimd.dma_start(out=_r(Dr[32 * g + 3:32 * g + 6, :]), in_=_r(Pg2s[:, :]))

            if not do_mlp:
                # est := gt (copies exercise the same f32r-output DVE/ACT path)
                nc.vector.scalar_tensor_tensor(out=_r(Pe[:, s, :]), in0=Pg[:, s, :],
                                               scalar=0.0, in1=Pg[:, s, :],
                                               op0=OP.add, op1=OP.bypass)
                nc.scalar.activation(_r(Pe2s[:, :]), Pe[:, s, :], ACTF.Square,
                                     accum_out=SES[:, s:s + 1])
                Br0, Cr0 = B_[s % 2], C_[s % 2]
                for g in range(2):
                    nc.gpsimd.dma_start(out=_r(Br0[32 * g:32 * g + 3, :]), in_=_r(Pe[:, s, :]))
                    nc.gpsimd.dma_start(out=_r(Cr0[32 * g:32 * g + 3, :]), in_=_r(Pe[:, s, :]))
                    nc.gpsimd.dma_start(out=_r(Cr0[32 * g + 3:32 * g + 6, :]), in_=_r(Pe2s[:, :]))
                for _ in range(9):
                    advance(hooks)
                return
            h1t = h1p.tile([128, 4, N], F32, tag="h1", name=f"h1_{s}")
            for c in range(4):
                ps = psA.tile([128, N], F32, tag="a", name=f"l1ps{s}_{c}")
                for j in range(2):
                    nc.tensor.matmul(ps[:, 512 * j:512 * (j + 1)],
                                     _r(w1o[:, 128 * c:128 * (c + 1)]),
                                     _r(obsT[:, 512 * j:512 * (j + 1)]),
                                     start=True, stop=True)
                nc.scalar.activation(_r(h1t[:, c, :]), ps[:, :], ACTF.Relu,
                                     bias=latb[:, c, s:s + 1])
                advance(hooks)

            h2t = h2p.tile([128, 4, N], F32, tag="h2", name=f"h2_{s}")
            for c in range(4):
                ps = psA.tile([128, N], F32, tag="a", name=f"l2ps{s}_{c}")
                for j in range(2):
                    for k in range(4):
                        nc.tensor.matmul(ps[:, 512 * j:512 * (j + 1)],
                                         _r(w2t[:, k, 128 * c:128 * (c + 1)]),
                                         _r(h1t[:, k, 512 * j:512 * (j + 1)]),
                                         start=(k == 0), stop=(k == 3))
                nc.scalar.activation(_r(h2t[:, c, :]), ps[:, :], ACTF.Relu,
                                     bias=b2t[:, c:c + 1])
                advance(hooks)

            h3t = h3p.tile([128, 2, N], F32, tag="h3", name=f"h3_{s}")
            for c in range(2):
                ps = psA.tile([128, N], F32, tag="a", name=f"l3ps{s}_{c}")
                for j in range(2):
                    for k in range(4):
                        nc.tensor.matmul(ps[:, 512 * j:512 * (j + 1)],
                                         _r(w3t[:, k, 128 * c:128 * (c + 1)]),
                                         _r(h2t[:, k, 512 * j:512 * (j + 1)]),
                                         start=(k == 0), stop=(k == 3))
                nc.scalar.activation(_r(h3t[:, c, :]), ps[:, :], ACTF.Relu,
                                     bias=b3t[:, c:c + 1])
                advance(hooks)

            ps4 = psG.tile([128, 1024], F32, tag="g", name=f"l4ps{s}")
            for j in range(2):
                for k in range(2):
                    nc.tensor.matmul(ps4[0:3, 512 * j:512 * (j + 1)],
                                     _r(w4t[:, k, :]),
                                     _r(h3t[:, k, 512 * j:512 * (j + 1)]),
                                     start=(k == 0), stop=(k == 1))
            nc.vector.scalar_tensor_tensor(out=_r(Pe[:, s, :]), in0=obsT[:, :],
                                           scalar=b4p[:, 0:1], in1=ps4[0:3, :],
                                           op0=OP.add, op1=OP.add)
            nc.scalar.activation(_r(Pe2s[:, :]), Pe[:, s, :], ACTF.Square,
                                 accum_out=SES[:, s:s + 1])
            Br, Cr = B_[s % 2], C_[s % 2]
            for g in range(2):
                nc.gpsimd.dma_start(out=_r(Br[32 * g:32 * g + 3, :]), in_=_r(Pe[:, s, :]))
                nc.gpsimd.dma_start(out=_r(Cr[32 * g:32 * g + 3, :]), in_=_r(Pe[:, s, :]))
                nc.gpsimd.dma_start(out=_r(Cr[32 * g + 3:32 * g + 6, :]), in_=_r(Pe2s[:, :]))
            advance(hooks)

        pending = None
        for s in range(BS):
            mlp(s, pending)
            if pending is not None:
                for _ in pending:
                    pass
            if do_gram:
                pending = gram_rounds(s)
        if pending is not None:
            for _ in pending:
                pass

        # ---------- finale ----------
        # cross term sum(gt*est) per coordinate -> Ft col 4 (in-place dump into Pe)
        nc.vector.scalar_tensor_tensor(out=Pe[:, :, :], in0=Pg[:, :, :],
                                       scalar=0.0, in1=Pe[:, :, :],
                                       op0=OP.add, op1=OP.mult,
                                       accum_out=Ft[0:3, 4:5])
        nc.vector.tensor_reduce(out=Ft[0:3, 2:3], in_=SGS[:, :], axis=AX.X, op=OP.add)
        nc.vector.tensor_reduce(out=Ft[0:3, 3:4], in_=SES[:, :], axis=AX.X, op=OP.add)
        nc.vector.tensor_reduce(out=Ft[:, 0:1], in_=M1[:, :], axis=AX.X, op=OP.add)
        nc.vector.tensor_reduce(out=Ft[:, 1:2], in_=M2[:, :], axis=AX.X, op=OP.add)

        fps = psG.tile([128, 1024], F32, tag="g", name="fps")
        nc.tensor.matmul(fps[0:1, 0:8], ones_c[:, :], Ft[:, :],
                         start=True, stop=True)
        nc.scalar.activation(outs[:, :], fps[0:1, 0:8], ACTF.Copy)
        nc.sync.dma_start(out=out_d, in_=outs)

    nc.compile()
    return nc


_program_cache = []


def kernel(**inputs):
    global LAST
    if not _program_cache:
        _program_cache.append(build_program())
    nc = _program_cache[0]

    def f32(x):
        return np.ascontiguousarray(np.asarray(x, dtype=np.float32))

    W1 = np.asarray(inputs["W1"], np.float32)
    W2 = np.asarray(inputs["W2"], np.float32)
    W3 = np.asarray(inputs["W3"], np.float32)
    W4 = np.asarray(inputs["W4"], np.float32)
    shared = {
        "w1o": f32(W1[0:3, :]),
        "w1l": f32(W1[3:259, :].reshape(2, 128, 512).transpose(1, 0, 2)),
        "b1r": f32(np.asarray(inputs["b1"], np.float32).reshape(1, 512)),
        "w2p": f32(W2.reshape(4, 128, 512).transpose(1, 0, 2)),
        "b2p": f32(np.asarray(inputs["b2"], np.float32).reshape(4, 128).T),
        "w3p": f32(W3.reshape(4, 128, 256).transpose(1, 0, 2)),
        "b3p": f32(np.asarray(inputs["b3"], np.float32).reshape(2, 128).T),
        "w4p": f32(W4.reshape(2, 128, 3).transpose(1, 0, 2)),
        "b4p": f32(np.asarray(inputs["b4"], np.float32).reshape(3, 1)),
    }
    in_maps = []
    for c in range(NCORES):
        sl = slice(c * BS, (c + 1) * BS)
        m = dict(shared)
        m["obs_t"] = f32(np.asarray(inputs["obs"][sl], np.float32).transpose(2, 0, 1))
        m["gt_t"] = f32(np.asarray(inputs["obs_gt"][sl], np.float32).transpose(2, 0, 1))
        m["lat_t"] = f32(np.asarray(inputs["latent"][sl], np.float32).T)
        in_maps.append(m)

    res = run_bass_kernel_spmd(nc, in_maps, core_ids=list(range(NCORES)),
                               trace=TRACE)
    LAST = res

    parts = np.stack([r["partials"][0] for r in res.results]).astype(np.float64)
    s_max1 = parts[:, 0].sum()
    s_max2 = parts[:, 1].sum()
    s_gt2 = parts[:, 2].sum()
    s_est2 = parts[:, 3].sum()
    s_cross = parts[:, 4].sum()
    chm = (s_gt2 - 2.0 * s_max1) / (B * N) + (s_est2 - 2.0 * s_max2) / (B * N)
    l2 = (s_gt2 - 2.0 * s_cross + s_est2) / (B * N * 3)
    loss = 0.2 * chm + 0.8 * l2
    return np.asarray(loss, dtype=np.float32)
